# revision 15
# baseline (speedup 1.0000x reference)
"""Viterbi CRF decode on Trainium2 (Bass), 8-core data-parallel.

Problem: B=128, S=512, T=32 (30 labels + START=30, END=31).
  forward max-plus scan over S steps, backpointers, masked lengths,
  backward pointer-following pass. Output [B, S] int32 tag path.

Sharding: pure data parallel, 16 examples per core.

Per-core layout (SBUF partitions p = 32*q + j, quadrant q in [0,4) holds
examples b = 4q+br, br in [0,4); j in [0,32) is the tag index):
  - state P4[p, (br,i)] = part[b, i] (part vector replicated across the 32
    j-partitions of each quadrant)
  - per step: scores = feats+trans (bcast APs), vals = scores + P4,
    segmented max-reduce over i -> part history PH[:, 4t+br],
    eq/iota-desc/max-reduce -> backpointer history (first-argmax encoded
    as 31-i), then a 32x32 block transpose + 4 broadcast stream_shuffles
    rebuild P4 for the next step.
  - pointer phase: arithmetic select of part at last valid position
    (monotone mask -> at-last indicator), argmax into END tag.
  - backward: per step, block-transpose of the bp row + one fused
    scalar_tensor_tensor (one-hot select, sum-accumulate) = the gather.

All compute on the vector engine (exact fp32, same association order as
the jax reference: (feats + trans) + part), DMA on sync engine.
"""

import numpy as np
from contextlib import ExitStack

import concourse.bass as bass
import concourse.mybir as mybir
from concourse.bass_utils import run_bass_kernel_spmd

F32 = mybir.dt.float32
I32 = mybir.dt.int32
AX = mybir.AxisListType
OP = mybir.AluOpType

T = 32
START = 30
END = 31
NCORES = 8


def build_nc(S, debug=False, reps=1):
    # Single compute engine (DVE) in program order: same-engine RAW/WAW is
    # serialized by the hardware (per-op pipe drain); the conservative race
    # detector does not model engine ordering, so it is disabled.
    nc = bass.Bass(detect_race_conditions=False)
    ft_d = nc.declare_dram_parameter("ft", [128, 4 * S], F32, isOutput=False)
    mkf_d = nc.declare_dram_parameter("mkf", [128, 4 * S + 4], F32, isOutput=False)
    tt_d = nc.declare_dram_parameter("tt", [128, 32], F32, isOutput=False)
    cst_d = nc.declare_dram_parameter("cst", [128, 64], F32, isOutput=False)
    dec_d = nc.declare_dram_parameter("dec", [128, S], I32, isOutput=True)
    if debug:
        ph_d = nc.declare_dram_parameter("d_ph", [128, 4 * S + 32], F32, isOutput=True)
        bpw_d = nc.declare_dram_parameter("d_bpw", [128, 4 * S + 32], F32, isOutput=True)
        bpf_d = nc.declare_dram_parameter("d_bpf", [128, 4 * S + 32], F32, isOutput=True)
        decf_d = nc.declare_dram_parameter("d_decf", [128, S], F32, isOutput=True)
        p4_d = nc.declare_dram_parameter("d_p4", [128, 128], F32, isOutput=True)
        p32_d = nc.declare_dram_parameter("d_p32", [128, 32], F32, isOutput=True)
        lpp_d = nc.declare_dram_parameter("d_lpp", [128, 32], F32, isOutput=True)

    K = S - 1  # bp rows k in [0, K)

    with ExitStack() as ctx:
        def sb(name, shape, dt=F32):
            return ctx.enter_context(nc.sbuf_tensor(name, shape, dt))

        FT = sb("FT", [128, 4 * S])
        MKF = sb("MKF", [128, 4 * S + 4])
        TT = sb("TT", [128, 32])
        PH = sb("PH", [128, 4 * S + 32])
        BPW = sb("BPW", [128, 4 * S + 32])
        XS = sb("XS", [128, 4 * S + 32])
        XS2 = sb("XS2", [128, 4 * S + 32])
        ALF = sb("ALF", [128, 4 * S])
        ALB = sb("ALB", [128, 4 * S])
        P4 = sb("P4", [128, 128])
        S4 = sb("S4", [128, 128])
        V = sb("V", [128, 128])
        T32 = sb("T32", [128, 32])
        DEC = sb("DEC", [128, S])
        DECI = sb("DECI", [128, S], I32)
        CST = sb("CST", [128, 64])
        TEND = sb("TEND", [128, 32])
        LPP = sb("LPP", [128, 32])
        TLP = sb("TLP", [128, 32])
        CAND = sb("CAND", [128, 32])
        MX = sb("MX", [128, 1])
        EQC = sb("EQC", [128, 32])
        PW = sb("PW", [128, 1])
        P32 = sb("P32", [128, 32])
        PR = sb("PR", [128, 32])
        SC = sb("SC", [128, 32])

        with (
            nc.semaphore() as dma_sem,
            nc.semaphore() as done_sem,
            nc.Block() as block,
        ):
            @block.sync
            def _(sync):
                sync.dma_start(out=FT[:], in_=ft_d[:]).then_inc(dma_sem, 16)
                sync.dma_start(out=MKF[:], in_=mkf_d[:]).then_inc(dma_sem, 16)
                sync.dma_start(out=TT[:], in_=tt_d[:]).then_inc(dma_sem, 16)
                sync.dma_start(out=CST[:], in_=cst_d[:]).then_inc(dma_sem, 16)
                sync.wait_ge(done_sem, 1)
                sync.dma_start(out=dec_d[:], in_=DECI[:]).then_inc(dma_sem, 16)
                if debug:
                    sync.dma_start(out=ph_d[:], in_=PH[:]).then_inc(dma_sem, 16)
                    sync.dma_start(out=bpw_d[:], in_=BPW[:]).then_inc(dma_sem, 16)
                    sync.dma_start(out=bpf_d[:], in_=XS[:]).then_inc(dma_sem, 16)
                    sync.dma_start(out=decf_d[:], in_=DEC[:]).then_inc(dma_sem, 16)
                    sync.dma_start(out=p4_d[:], in_=P4[:]).then_inc(dma_sem, 16)
                    sync.dma_start(out=p32_d[:], in_=P32[:]).then_inc(dma_sem, 16)
                    sync.dma_start(out=lpp_d[:], in_=LPP[:]).then_inc(dma_sem, 16)

            def emit_body(v):
                # constants / scratch init
                v.stream_shuffle(out=TEND[:], in_=TT[:], mask=[END] * 32)
                v.memset(PH[:], 0.0)
                v.memset(XS[:, 4 * K:], 0.0)
                v.memset(BPW[:, 4 * K:], 0.0)
                v.memset(P32[:], 0.0)
                v.memset(LPP[:], 0.0)

                # init t=0: part0[b, j] = feats[b,0,j] + trans[START, j]
                v.tensor_scalar_add(out=PH[:, 0:4], in0=FT[:, 0:4],
                                    scalar1=TT[:, START:START + 1])
                # independent fillers: give the PH write time to land before
                # the transpose reads it (HW has no end-write->read interlock)
                v.tensor_sub(out=ALF[:], in0=MKF[:, 0:4 * S], in1=MKF[:, 4:4 * S + 4])
                v.tensor_scalar(out=ALB[:], in0=ALF[:], scalar1=1.0,
                                scalar2=1e30, op0=OP.subtract, op1=OP.mult)
                v.transpose(out=T32[:], in_=PH[:, 0:32])
                for br in range(4):
                    v.stream_shuffle(out=P4[:, 32 * br:32 * br + 32],
                                     in_=T32[:], mask=[br] * 32)

                VB = [V, S4]  # double-buffered vals: V_t and V_{t-1}
                EB = XS2      # eq scratch (XS2 free until scatter phase)
                tt_b = TT[:].unsqueeze(1).broadcast_to([128, 4, 32])
                iotad_b = CST[:, 32:64].unsqueeze(1).broadcast_to([128, 4, 32])

                def bp_chain(tp):
                    # backpointer extraction for step tp (PH[tp] is >=3 ops old)
                    vp = VB[tp % 2][:]
                    vp3 = vp.rearrange("p (b i) -> p b i", b=4)
                    php = PH[:, 4 * tp:4 * tp + 4].unsqueeze(2).broadcast_to([128, 4, 32])
                    ev = EB[:, 0:128].rearrange("p (b i) -> p b i", b=4)
                    v.tensor_tensor(out=ev, in0=vp3, in1=php, op=OP.is_equal)
                    v.tensor_tensor(out=vp3, in0=ev, in1=iotad_b, op=OP.mult)
                    v.tensor_reduce(out=BPW[:, 4 * (tp - 1):4 * (tp - 1) + 4],
                                    in_=vp3, axis=AX.X, op=OP.max)

                # forward scan, bp-chain pipelined one step behind
                for t in range(1, S):
                    vc = VB[t % 2][:]
                    vc3 = vc.rearrange("p (b i) -> p b i", b=4)
                    ft_b = FT[:, 4 * t:4 * t + 4].unsqueeze(2).broadcast_to([128, 4, 32])
                    v.tensor_tensor(out=vc3, in0=ft_b, in1=tt_b, op=OP.add)
                    v.tensor_tensor(out=vc, in0=vc, in1=P4[:], op=OP.add)
                    v.tensor_reduce(out=PH[:, 4 * t:4 * t + 4], in_=vc3,
                                    axis=AX.X, op=OP.max)
                    if t > 1:
                        bp_chain(t - 1)
                    else:
                        v.drain()
                        v.drain()
                    if t < S - 1:
                        v.transpose(out=T32[:], in_=PH[:, 4 * t:4 * t + 32])
                        for br in range(4):
                            v.stream_shuffle(out=P4[:, 32 * br:32 * br + 32],
                                             in_=T32[:], mask=[br] * 32)
                bp_chain(S - 1)

                # last_partition by-i-partition: max over t of PH + ALB
                ph_bt = PH[:, 0:4 * S].rearrange("p (t b) -> p b t", b=4)
                alb_bt = ALB[:].rearrange("p (t b) -> p b t", b=4)
                xs_bt = XS[:, 0:4 * S].rearrange("p (t b) -> p b t", b=4)
                v.tensor_tensor(out=xs_bt, in0=ph_bt, in1=alb_bt, op=OP.add)
                v.tensor_reduce(out=LPP[:, 0:4], in_=xs_bt, axis=AX.X, op=OP.max)

                # bp decode + mask (independent of LPP; also serves as filler)
                v.tensor_scalar(out=XS2[:, 0:4 * K], in0=BPW[:, 0:4 * K],
                                scalar1=-1.0, scalar2=31.0, op0=OP.mult, op1=OP.add)
                v.tensor_tensor(out=BPW[:, 0:4 * K], in0=XS2[:, 0:4 * K],
                                in1=MKF[:, 4:4 * K + 4], op=OP.mult)

                # pointer = argmax_i(LP[b,i] + trans[i,END]); one-time tail,
                # explicit drains around every end-write -> start-read pair
                v.transpose(out=TLP[:], in_=LPP[:])
                v.drain()
                v.tensor_tensor(out=CAND[:], in0=TLP[:], in1=TEND[:], op=OP.add)
                v.tensor_reduce(out=MX[:], in_=CAND[:], axis=AX.X, op=OP.max)
                v.drain()
                v.tensor_tensor(out=EQC[:], in0=CAND[:],
                                in1=MX[:].broadcast_to([128, 32]), op=OP.is_equal)
                v.tensor_tensor(out=SC[:], in0=EQC[:], in1=CST[:, 32:64], op=OP.mult)
                v.tensor_reduce(out=PW[:], in_=SC[:], axis=AX.X, op=OP.max)
                v.drain()
                v.tensor_scalar(out=P32[:, 0:1], in0=PW[:], scalar1=-1.0,
                                scalar2=31.0, op0=OP.mult, op1=OP.add)
                v.drain()

                # scatter pointer at k == last_pos: bp' = bp + atlast*(ptr - bp)
                v.transpose(out=T32[:], in_=P32[:])
                v.stream_shuffle(out=PR[:], in_=T32[:], mask=[0] * 32)
                v.drain()
                pr_b = PR[:, 0:4].unsqueeze(1).broadcast_to([128, K, 4])
                bp_v = BPW[:, 0:4 * K].rearrange("p (k b) -> p k b", b=4)
                xs_v = XS[:, 0:4 * K].rearrange("p (k b) -> p k b", b=4)
                xs2_v = XS2[:, 0:4 * K].rearrange("p (k b) -> p k b", b=4)
                alf_v = ALF[:, 0:4 * K].rearrange("p (k b) -> p k b", b=4)
                v.tensor_tensor(out=xs_v, in0=pr_b, in1=bp_v, op=OP.subtract)
                v.tensor_tensor(out=xs2_v, in0=xs_v, in1=alf_v, op=OP.mult)
                v.tensor_tensor(out=xs_v, in0=bp_v, in1=xs2_v, op=OP.add)

                # backward pass: transposes pipelined 2 steps ahead of the stt
                v.tensor_copy(out=DEC[:, S - 1:S], in_=P32[:, 0:1])
                TB = [T32, TLP, SC]  # ring of transpose buffers
                v.transpose(out=TB[(S - 2) % 3][:], in_=XS[:, 4 * (S - 2):4 * (S - 2) + 32])
                v.transpose(out=TB[(S - 3) % 3][:], in_=XS[:, 4 * (S - 3):4 * (S - 3) + 32])
                for k in range(S - 2, -1, -1):
                    if k >= 2:
                        v.transpose(out=TB[(k - 2) % 3][:],
                                    in_=XS[:, 4 * (k - 2):4 * (k - 2) + 32])
                    else:
                        v.drain()
                    v.scalar_tensor_tensor(out=EQC[:], in0=CST[:, 0:32],
                                           scalar=DEC[:, k + 1:k + 2],
                                           in1=TB[k % 3][:],
                                           op0=OP.is_equal, op1=OP.mult,
                                           accum_out=DEC[:, k:k + 1])

                v.drain()
                v.tensor_copy(out=DECI[:], in_=DEC[:])

            @block.vector
            def _(v):
                v.wait_ge(dma_sem, 64)
                for _rep in range(reps):
                    emit_body(v)
                v.drain().then_inc(done_sem, 1)

    return nc


def pack_inputs(feats, transitions, mask, S):
    """Host-side layout packing (pure data movement, no arithmetic beyond
    dtype conversion of the 0/1 mask)."""
    trans = np.ascontiguousarray(np.asarray(transitions, np.float32))
    ttrep = np.ascontiguousarray(np.tile(trans.T, (4, 1)))  # [128, 32]
    iota = np.arange(32, dtype=np.float32)
    cst = np.ascontiguousarray(
        np.tile(np.concatenate([iota, 31.0 - iota])[None, :], (128, 1)))
    in_maps = []
    bc = 16
    for c in range(NCORES):
        f = np.asarray(feats[bc * c:bc * c + bc], np.float32)  # [16, S, 32]
        ft = np.ascontiguousarray(
            f.reshape(4, 4, S, T).transpose(0, 3, 2, 1).reshape(128, 4 * S))
        m = np.asarray(mask[bc * c:bc * c + bc]).astype(np.float32)  # [16, S]
        mk = np.broadcast_to(
            m.reshape(4, 1, 4, S).transpose(0, 1, 3, 2), (4, 32, S, 4))
        mk = mk.reshape(128, 4 * S)
        mkp = np.zeros((128, 4 * S + 4), np.float32)
        mkp[:, :4 * S] = mk
        in_maps.append({"ft": ft, "mkf": mkp, "tt": ttrep, "cst": cst})
    return in_maps


def unpack_outputs(results, S):
    out = np.empty((128, S), np.int32)
    bc = 16
    for c in range(NCORES):
        d = np.asarray(results[c]["dec"]).reshape(4, 32, S)
        out[bc * c:bc * c + bc] = d[:, 0:4, :].reshape(16, S)
    return out


_NC_CACHE = {}


def kernel(feats, transitions, mask):
    B, S, Tin = feats.shape
    assert (B, Tin) == (128, 32)
    if S not in _NC_CACHE:
        _NC_CACHE[S] = build_nc(S)
    nc = _NC_CACHE[S]
    in_maps = pack_inputs(feats, transitions, mask, S)
    res = run_bass_kernel_spmd(nc, in_maps, list(range(NCORES)))
    return unpack_outputs(res.results, S)
